# revision 20
# baseline (speedup 1.0000x reference)
"""Trainium2 Bass kernel for the autoregressive TCN decoder (nn_Decoder).

Strategy
--------
Pure batch data-parallel: B=64 is split 8 ways, each NeuronCore runs the
full T=32 autoregressive decode for its 8 batch rows; no collectives.

Instead of the reference's O(T^2) re-run of the whole TCN on a growing
prefix, we decode incrementally (exact, since the convs are causal): per
step each dilated conv needs only 3 matmul taps against cached per-layer
histories.

Layout: channels on SBUF partitions, batch (8) on the free dim.  Each
conv tap is matmul(psum, lhsT=W[k] [Cin,Cout], rhs=x_hist[:,pos] [Cin,8])
accumulating in PSUM.  Residual adds are folded into the PSUM group via
an identity-matrix matmul.  ELU is computed exactly as
    elu(z) + 1 = min(exp(z), max(z+1, 1))
(one ScalarE Exp + one fused VectorE tensor_scalar + one tensor_tensor),
tracking shifted activations x~ = elu(z)+1 so the "+1" is absorbed into
per-layer bias corrections  corr = bias - sum_present_taps colsum(W_k),
which the kernel computes on device in a short prologue (ones-vector
matmuls).  Causal zero-padding at early steps is handled by skipping
absent taps and selecting the matching tap-mask correction column.
"""
import os
import sys
import functools
import numpy as np

sys.path.insert(0, "/opt/trn_rl_repo")

import concourse.bass as bass
import concourse.bacc as bacc
import concourse.mybir as mybir
from concourse.tile import TileContext
from concourse.bass_utils import run_bass_kernel_spmd

F32 = mybir.dt.float32
AF = mybir.ActivationFunctionType
OP = mybir.AluOpType

# architecture constants (hardcoded per problem spec)
T = 32
B = 64
NCORES = 8
BC = B // NCORES          # 8 batch rows per core
F = 128
CIN = 33
NB = 8
DILS = [1, 2, 4, 8, 1, 2, 4, 8]
NLAY = 15                 # corrected conv layers: conv2_0, (conv1_b, conv2_b) b=1..7
NCOL = 50                 # correction columns: 45 layer-mask + 3 head + b_res + b_in

LAST_EXEC_NS = None


def _lay_w(lay):
    """(which weight tile, block index) for corrected conv layer `lay`."""
    if lay == 0:
        return ("w2", 0)
    b = (lay + 1) // 2
    if lay % 2 == 1:
        return ("w1", b)
    return ("w2", b)


def build_nc(reps=1, f32r=False):
    nc = bacc.Bacc("TRN2", target_bir_lowering=False, debug=False)

    xdec_d = nc.declare_dram_parameter("xdec", [32, T * BC], F32, isOutput=False)
    ylast_d = nc.declare_dram_parameter("ylast", [1, BC], F32, isOutput=False)
    win_d = nc.declare_dram_parameter("win", [CIN, 3 * F], F32, isOutput=False)
    wres_d = nc.declare_dram_parameter("wres", [CIN, F], F32, isOutput=False)
    w1_d = nc.declare_dram_parameter("w1l", [F, 7 * 3 * F], F32, isOutput=False)
    w2_d = nc.declare_dram_parameter("w2l", [F, 8 * 3 * F], F32, isOutput=False)
    wd0t_d = nc.declare_dram_parameter("wd0t", [F, F], F32, isOutput=False)
    wd0b_d = nc.declare_dram_parameter("wd0b", [F, F], F32, isOutput=False)
    wd1_d = nc.declare_dram_parameter("wd1", [F, 64], F32, isOutput=False)
    wd2_d = nc.declare_dram_parameter("wd2", [64, 1], F32, isOutput=False)
    encl_d = nc.declare_dram_parameter("encl", [F, BC], F32, isOutput=False)
    biasE_d = nc.declare_dram_parameter("biasE", [F, NCOL], F32, isOutput=False)
    biasT_d = nc.declare_dram_parameter("biasT", [F, NCOL], F32, isOutput=False)
    ident_d = nc.declare_dram_parameter("ident", [F, F], F32, isOutput=False)
    out_d = nc.declare_dram_parameter("out", [1, T * BC], F32, isOutput=True)

    with TileContext(nc) as tc:
        with (
            tc.tile_pool(name="consts", bufs=1) as cpool,
            tc.tile_pool(name="steps", bufs=3) as spool,
            tc.tile_pool(name="ps", bufs=1, space="PSUM") as ppool,
        ):
            # ---- const SBUF tiles + input DMAs ----
            win_sb = cpool.tile([CIN, 3 * F], F32, name="win_sb")
            wres_sb = cpool.tile([CIN, F], F32, name="wres_sb")
            w1_sb = cpool.tile([F, 7 * 3 * F], F32, name="w1_sb")
            w2_sb = cpool.tile([F, 8 * 3 * F], F32, name="w2_sb")
            wd0t_sb = cpool.tile([F, F], F32, name="wd0t_sb")
            wd0b_sb = cpool.tile([F, F], F32, name="wd0b_sb")
            wd1_sb = cpool.tile([F, 64], F32, name="wd1_sb")
            wd2_sb = cpool.tile([64, 1], F32, name="wd2_sb")
            encl_sb = cpool.tile([F, BC], F32, name="encl_sb")
            biasE_sb = cpool.tile([F, NCOL], F32, name="biasE_sb")
            biasT_sb = cpool.tile([F, NCOL], F32, name="biasT_sb")
            ident_sb = cpool.tile([F, F], F32, name="ident_sb")
            ones_sb = cpool.tile([F, 1], F32, name="ones_sb")
            csblk = cpool.tile([F, NCOL], F32, name="csblk")
            corrE = cpool.tile([F, NCOL], F32, name="corrE")
            corrT = cpool.tile([F, NCOL], F32, name="corrT")
            encc = cpool.tile([F, BC], F32, name="encc")
            dummy = cpool.tile([1, 1], F32, name="dummy")
            cimm = cpool.tile([F, 2], F32, name="cimm")

            # persistent histories
            x_hist = cpool.tile([CIN, (T + 1) * BC], F32, name="x_hist")
            ctils = [cpool.tile([F, T * BC], F32, name=f"ctil{b}") for b in range(NB)]
            htils = [cpool.tile([F, T * BC], F32, name=f"htil{b}") for b in range(NB)]

            # chunk big weight DMAs to 128-col pieces: keeps the per-reader
            # semaphore wait count below the hardware matmul wait-slot limit
            for sb, dr in [
                (biasE_sb, biasE_d), (biasT_sb, biasT_d),
                (win_sb, win_d), (wres_sb, wres_d),
                (w1_sb, w1_d), (w2_sb, w2_d),
                (wd0t_sb, wd0t_d), (wd0b_sb, wd0b_d),
                (wd1_sb, wd1_d), (wd2_sb, wd2_d),
                (encl_sb, encl_d), (ident_sb, ident_d),
            ]:
                ncols = sb.shape[-1]
                for c0 in range(0, ncols, F):
                    c1 = min(c0 + F, ncols)
                    nc.sync.dma_start(sb[:, c0:c1], dr[:, c0:c1])
            nc.sync.dma_start(x_hist[0:32, 0:T * BC], xdec_d[:, :])
            nc.sync.dma_start(x_hist[32:33, 0:BC], ylast_d[:, :])

            # warm the Exp table during DMA wait
            nc.scalar.activation(dummy[:, :], biasE_sb[0:1, 0:1], AF.Exp)
            nc.vector.memset(ones_sb[:, :], 1.0)
            nc.vector.memset(csblk[:, :], 0.0)
            nc.vector.memset(cimm[:, 0:1], -1.0)
            nc.vector.memset(cimm[:, 1:2], -2.0)

            # PE warm-up touches: one tiny matmul per DMA'd chunk / memset
            # tile, so each later matmul carries at most ONE sync wait
            # (the hardware LDWEIGHTS struct has very few wait slots).
            pswu = ppool.tile([1, 1], F32, name="pswu", tag="psy", bufs=1)
            touch_list = []
            for sb in (win_sb, wres_sb, w1_sb, w2_sb, wd0t_sb, wd0b_sb,
                       wd1_sb, wd2_sb, encl_sb, biasE_sb, biasT_sb,
                       ident_sb):
                p = sb.shape[0]
                for c0 in range(0, sb.shape[-1], F):
                    touch_list.append(sb[0:p, c0:c0 + 1])
            touch_list.append(x_hist[0:32, 0:1])
            touch_list.append(x_hist[0:32, 128:129])
            touch_list.append(x_hist[32:33, 0:1])
            touch_list.append(ones_sb[:, 0:1])
            wu_sb = cpool.tile([F, 1], F32, name="wu_sb")
            for t_ap in touch_list:
                nc.tensor.matmul(pswu[:, :], t_ap, t_ap, start=True, stop=True)
                nc.vector.tensor_copy(wu_sb[0:t_ap.shape[0], :], t_ap)

            # ---- prologue: colsum corrections ----
            psc = ppool.tile([F, 48], F32, name="psc", tag="psA", bufs=2)
            for lay in range(NLAY):
                wkind, b = _lay_w(lay)
                wt = w1_sb if wkind == "w1" else w2_sb
                base = ((b - 1) if wkind == "w1" else b) * 3
                for m in range(3):
                    ks = list(range(2 - m, 3))
                    col = lay * 3 + m
                    for j, k in enumerate(ks):
                        nc.tensor.matmul(
                            psc[:, col:col + 1],
                            wt[:, (base + k) * F:(base + k + 1) * F],
                            ones_sb[:, :],
                            start=(j == 0), stop=(j == len(ks) - 1),
                        )
            nc.tensor.matmul(psc[:, 45:46], wd0t_sb[:, :], ones_sb[:, :], start=True, stop=True)
            nc.tensor.matmul(psc[0:64, 46:47], wd1_sb[:, :], ones_sb[:, :], start=True, stop=True)
            nc.tensor.matmul(psc[0:1, 47:48], wd2_sb[:, :], ones_sb[0:64, :], start=True, stop=True)
            nc.vector.tensor_copy(csblk[:, 0:46], psc[:, 0:46])
            nc.vector.tensor_copy(csblk[0:64, 46:47], psc[0:64, 46:47])
            nc.vector.tensor_copy(csblk[0:1, 47:48], psc[0:1, 47:48])

            nc.vector.tensor_tensor(corrE[:, :], biasE_sb[:, :], csblk[:, :], op=OP.subtract)
            nc.vector.tensor_tensor(corrT[:, :], biasT_sb[:, :], csblk[:, :], op=OP.subtract)

            # enc_contrib = wd0b.T @ enc_last.T   [F, BC], constant over steps
            pse = ppool.tile([F, BC], F32, name="pse", tag="psA", bufs=2)
            nc.tensor.matmul(pse[:, :], wd0b_sb[:, :], encl_sb[:, :], start=True, stop=True)
            nc.vector.tensor_copy(encc[:, :], pse[:, :])

            # ---- helper: conv ELU site  out = min(exp(z), max(z+1,1)),
            # z = psum + corr[col]  (corr folds bias and colsum shifts) ----
            def elu_site(ps, col, out_ap, p=F):
                e_t = spool.tile([F, BC], F32, name="e_t", tag="e_t")
                t_t = spool.tile([F, BC], F32, name="t_t", tag="t_t")
                nc.scalar.activation(e_t[0:p, :], ps[0:p, :], AF.Exp,
                                     bias=corrE[0:p, col:col + 1])
                nc.vector.tensor_scalar(t_t[0:p, :], ps[0:p, :],
                                        corrT[0:p, col:col + 1], 1.0,
                                        op0=OP.add, op1=OP.max)
                nc.vector.tensor_tensor(out_ap, e_t[0:p, :], t_t[0:p, :], op=OP.min)

            # ---- helper: residual ELU site.  z = a + b + off (immediate);
            # out = elu(z)+1.  s = a+b on DVE; exp(s+off); max(s+off+1, 1). ----
            def elu_res_site(a_ap, b_ap, off, out_ap):
                s_t = spool.tile([F, BC], F32, name="s_t", tag="s_t")
                e_t = spool.tile([F, BC], F32, name="e_t", tag="e_t")
                t_t = spool.tile([F, BC], F32, name="t_t", tag="t_t")
                nc.vector.tensor_tensor(s_t[:, :], a_ap, b_ap, op=OP.add)
                bias_ap = cimm[:, 0:1] if off == -1.0 else cimm[:, 1:2]
                nc.scalar.activation(e_t[:, :], s_t[:, :], AF.Exp, bias=bias_ap)
                if off + 1.0 == 0.0:
                    nc.vector.tensor_scalar(t_t[:, :], s_t[:, :], 1.0, None,
                                            op0=OP.max)
                else:
                    nc.vector.tensor_scalar(t_t[:, :], s_t[:, :], off + 1.0, 1.0,
                                            op0=OP.add, op1=OP.max)
                nc.vector.tensor_tensor(out_ap, e_t[:, :], t_t[:, :], op=OP.min)

            # ---- main autoregressive loop (fully unrolled) ----
            # Conv weight lookup: (sbuf tile, column base) per (kind, block).
            def wsl(kind, b, k):
                if kind == "c1":
                    if b == 0:
                        return win_sb[:, k * F:(k + 1) * F]
                    return w1_sb[:, ((b - 1) * 3 + k) * F:((b - 1) * 3 + k + 1) * F]
                return w2_sb[:, (b * 3 + k) * F:(b * 3 + k + 1) * F]

            def hist(kind, b):
                if kind == "c1":
                    return x_hist if b == 0 else htils[b - 1]
                return ctils[b]

            F32R = mybir.dt.float32r

            def mmop(ps, lhsT, rhs, start, stop):
                if f32r:
                    lhsT = lhsT.bitcast(F32R)
                    rhs = rhs.bitcast(F32R)
                nc.tensor.matmul(ps[:, :], lhsT, rhs, start=start, stop=stop)

            # The final tap (k=2, current position) sits on the critical
            # path; taps k<2 read only prior-step history, so they are
            # pre-issued one block ahead to run during ELU waits.
            def old_taps(ps, kind, b, i):
                d = DILS[b]
                ntaps = 1 + (i >= d) + (i >= 2 * d)
                h = hist(kind, b)
                for j, k in enumerate(range(3 - ntaps, 2)):
                    p = i - (2 - k) * d
                    mmop(ps, wsl(kind, b, k), h[:, p * BC:(p + 1) * BC],
                         (j == 0), False)
                return ntaps

            def last_tap(ps, kind, b, i, ntaps, rhs):
                mmop(ps, wsl(kind, b, 2), rhs, (ntaps == 1), True)

            for i in list(range(T)) * reps:
                # -- pre-issue at step start: everything not needing y_hat(i-1)
                ps0 = ppool.tile([F, BC], F32, name="ps0", tag="ps0", bufs=1)
                nc.tensor.matmul(ps0[:, :], ident_sb[:, :], encc[:, :],
                                 start=True, stop=False)
                psB = ppool.tile([F, BC], F32, name="psB", tag="psB", bufs=2)
                ntB = old_taps(psB, "c2", 0, i)
                psA_next = ppool.tile([F, BC], F32, name="psA", tag="psA", bufs=2)
                ntA_next = old_taps(psA_next, "c1", 1, i)

                # -- block 0: conv_in + res (need y_hat from prev step) --
                d = DILS[0]
                ntaps = 1 + (i >= d) + (i >= 2 * d)
                ks = list(range(3 - ntaps, 3))
                psA = ppool.tile([F, BC], F32, name="psA", tag="psA", bufs=2)
                for j, k in enumerate(ks):
                    p = i - (2 - k) * d
                    nc.tensor.matmul(psA[:, :],
                                     win_sb[:, k * F:(k + 1) * F],
                                     x_hist[:, p * BC:(p + 1) * BC],
                                     start=(j == 0), stop=(j == len(ks) - 1))
                psR = ppool.tile([F, BC], F32, name="psR", tag="psR", bufs=1)
                nc.tensor.matmul(psR[:, :], wres_sb[:, :],
                                 x_hist[:, i * BC:(i + 1) * BC],
                                 start=True, stop=True)
                res_sb = spool.tile([F, BC], F32, name="res_sb", tag="res_sb")
                nc.vector.tensor_scalar(res_sb[:, :], psR[:, :],
                                        corrE[:, 48:49], None, op0=OP.add)
                elu_site(psA, 49, ctils[0][:, i * BC:(i + 1) * BC])

                for b in range(NB):
                    # psB for this block already holds its old taps; finish it.
                    last_tap(psB, "c2", b, i, ntB,
                             ctils[b][:, i * BC:(i + 1) * BC])
                    psB_cur, ntB_cur = psB, ntB
                    # pre-issue next block's old taps (fill ELU-B wait)
                    if b + 1 < NB:
                        psB = ppool.tile([F, BC], F32, name="psB", tag="psB", bufs=2)
                        ntB = old_taps(psB, "c2", b + 1, i)
                    lay = 0 if b == 0 else 2 * b
                    c2_t = spool.tile([F, BC], F32, name="c2_t", tag="c2_t")
                    elu_site(psB_cur, lay * 3 + (ntB_cur - 1), c2_t[:, :])
                    if b == 0:
                        elu_res_site(c2_t[:, :], res_sb[:, :], -1.0,
                                     htils[0][:, i * BC:(i + 1) * BC])
                    else:
                        elu_res_site(c2_t[:, :],
                                     htils[b - 1][:, i * BC:(i + 1) * BC], -2.0,
                                     htils[b][:, i * BC:(i + 1) * BC])
                    if b + 1 < NB:
                        # conv1 of block b+1: finish on-path tap
                        psA_cur, ntA_cur = psA_next, ntA_next
                        last_tap(psA_cur, "c1", b + 1, i, ntA_cur,
                                 htils[b][:, i * BC:(i + 1) * BC])
                        if b + 2 < NB:
                            psA_next = ppool.tile([F, BC], F32, name="psA",
                                                  tag="psA", bufs=2)
                            ntA_next = old_taps(psA_next, "c1", b + 2, i)
                        lay = 2 * (b + 1) - 1
                        elu_site(psA_cur, lay * 3 + (ntA_cur - 1),
                                 ctils[b + 1][:, i * BC:(i + 1) * BC])

                # --- head ---
                nc.tensor.matmul(ps0[:, :], wd0t_sb[:, :],
                                 htils[NB - 1][:, i * BC:(i + 1) * BC],
                                 start=False, stop=True)
                o0_t = spool.tile([F, BC], F32, name="o0_t", tag="o0_t")
                elu_site(ps0, 45, o0_t[:, :])

                ps1 = ppool.tile([64, BC], F32, name="ps1", tag="ps1", bufs=1)
                nc.tensor.matmul(ps1[:, :], wd1_sb[:, :], o0_t[:, :],
                                 start=True, stop=True)
                o1_t = spool.tile([64, BC], F32, name="o1_t", tag="o1_t")
                elu_site(ps1, 46, o1_t[:, :], p=64)

                psy = ppool.tile([1, BC], F32, name="psy", tag="psy", bufs=1)
                nc.tensor.matmul(psy[:, :], wd2_sb[:, :], o1_t[:, :],
                                 start=True, stop=True)
                nc.vector.tensor_scalar(
                    x_hist[32:33, (i + 1) * BC:(i + 2) * BC], psy[:, :],
                    corrE[0:1, 47:48], None, op0=OP.add)

            nc.sync.dma_start(out_d[:, :], x_hist[32:33, BC:(T + 1) * BC])

    nc.compile()
    return nc


def prepare_in_maps(inputs):
    f32 = np.float32

    def a(x):
        return np.ascontiguousarray(np.asarray(x, f32))

    dec = a(inputs["data_decoder"])          # [B,T,32]
    ly = a(inputs["last_y"])                 # [B]
    enc = a(inputs["data_encoder"])          # [B,128,128]
    w_in = a(inputs["w_in"])                 # [3,33,128]
    b_in = a(inputs["b_in"])
    w_res = a(inputs["w_res"])               # [1,33,128]
    b_res = a(inputs["b_res"])
    w1 = a(inputs["w1"])                     # [7,3,128,128]
    b1 = a(inputs["b1"])
    w2 = a(inputs["w2"])                     # [8,3,128,128]
    b2 = a(inputs["b2"])
    wd0 = a(inputs["wd0"])                   # [256,128]
    bd0 = a(inputs["bd0"])
    wd1 = a(inputs["wd1"])                   # [128,64]
    bd1 = a(inputs["bd1"])
    wd2 = a(inputs["wd2"])                   # [64,1]
    bd2 = a(inputs["bd2"])

    win_l = np.ascontiguousarray(w_in.transpose(1, 0, 2).reshape(CIN, 3 * F))
    w1_l = np.ascontiguousarray(w1.transpose(2, 0, 1, 3).reshape(F, 7 * 3 * F))
    w2_l = np.ascontiguousarray(w2.transpose(2, 0, 1, 3).reshape(F, 8 * 3 * F))

    biasE = np.zeros((F, NCOL), f32)
    biasT = np.zeros((F, NCOL), f32)
    for lay in range(NLAY):
        if lay % 2 == 1:
            bb = b1[(lay + 1) // 2 - 1]
        else:
            bb = b2[lay // 2]
        for m in range(3):
            biasE[:, lay * 3 + m] = bb
            biasT[:, lay * 3 + m] = bb + 1.0
    biasE[:, 45], biasT[:, 45] = bd0, bd0 + 1.0
    biasE[:64, 46], biasT[:64, 46] = bd1, bd1 + 1.0
    biasE[0, 47] = bd2[0]
    biasE[:, 48] = b_res
    biasE[:, 49], biasT[:, 49] = b_in, b_in + 1.0

    ident = np.eye(F, dtype=f32)

    in_maps = []
    for c in range(NCORES):
        sl = slice(c * BC, (c + 1) * BC)
        # xdec[ch, i*BC + j] = dec[batch j, step i, ch]
        xdec = np.ascontiguousarray(dec[sl].transpose(2, 1, 0).reshape(32, T * BC))
        ylast = np.ascontiguousarray(ly[sl].reshape(1, BC))
        encl = np.ascontiguousarray(enc[sl, -1, :].T)       # [128, BC]
        in_maps.append({
            "xdec": xdec, "ylast": ylast,
            "win": win_l, "wres": np.ascontiguousarray(w_res[0]),
            "w1l": w1_l, "w2l": w2_l,
            "wd0t": np.ascontiguousarray(wd0[:F]),
            "wd0b": np.ascontiguousarray(wd0[F:]),
            "wd1": wd1, "wd2": wd2,
            "encl": encl, "biasE": biasE, "biasT": biasT,
            "ident": ident,
        })
    return in_maps


@functools.lru_cache(maxsize=1)
def _built_nc():
    return build_nc()


def kernel(**inputs) -> np.ndarray:
    global LAST_EXEC_NS
    nc = _built_nc()
    in_maps = prepare_in_maps(inputs)
    trace = bool(os.environ.get("KERNEL_TRACE"))
    try:
        r = run_bass_kernel_spmd(nc, in_maps, list(range(NCORES)), trace=trace)
    except ModuleNotFoundError:
        r = run_bass_kernel_spmd(nc, in_maps, list(range(NCORES)), trace=False)
    LAST_EXEC_NS = r.exec_time_ns if r.exec_time_ns else r.mean_exec_time_ns
    outs = []
    for c in range(NCORES):
        o = np.asarray(r.results[c]["out"]).reshape(T, BC).T   # [BC, T]
        outs.append(o)
    return np.ascontiguousarray(np.concatenate(outs, axis=0).astype(np.float32))


# revision 21
# speedup vs baseline: 316.3333x; 316.3333x over previous
"""Trainium2 Bass kernel for the autoregressive TCN decoder (nn_Decoder).

Strategy
--------
Pure batch data-parallel: B=64 is split 8 ways, each NeuronCore runs the
full T=32 autoregressive decode for its 8 batch rows; no collectives.

Instead of the reference's O(T^2) re-run of the whole TCN on a growing
prefix, we decode incrementally (exact, since the convs are causal): per
step each dilated conv needs only 3 matmul taps against cached per-layer
histories.

Layout: channels on SBUF partitions, batch (8) on the free dim.  Each
conv tap is matmul(psum, lhsT=W[k] [Cin,Cout], rhs=x_hist[:,pos] [Cin,8])
accumulating in PSUM.  The constant encoder contribution to the head is
folded into its PSUM group via an identity-matrix matmul.  ELU is
computed exactly as
    elu(z) + 1 = min(exp(z), max(z+1, 1))
(one ScalarE Exp + one fused VectorE tensor_scalar + one tensor_tensor),
tracking shifted activations x~ = elu(z)+1 so the "+1" is absorbed into
per-layer bias corrections  corr = bias - sum_present_taps colsum(W_k),
which the kernel computes on device in a short prologue (ones-vector
matmuls).  Causal zero-padding at early steps is handled by skipping
absent taps and selecting the matching tap-mask correction column.
"""
import os
import sys
import functools
import numpy as np

sys.path.insert(0, "/opt/trn_rl_repo")

import concourse.bass as bass
import concourse.bacc as bacc
import concourse.mybir as mybir
from concourse.tile import TileContext
from concourse.bass_utils import run_bass_kernel_spmd

F32 = mybir.dt.float32
AF = mybir.ActivationFunctionType
OP = mybir.AluOpType

# architecture constants (hardcoded per problem spec)
T = 32
B = 64
NCORES = 8
BC = B // NCORES          # 8 batch rows per core
F = 128
CIN = 33
NB = 8
DILS = [1, 2, 4, 8, 1, 2, 4, 8]
NLAY = 15                 # corrected conv layers: conv2_0, (conv1_b, conv2_b) b=1..7
NCOL = 50                 # correction columns: 45 layer-mask + 3 head + b_res + b_in

LAST_EXEC_NS = None


def _lay_w(lay):
    """(which weight tile, block index) for corrected conv layer `lay`."""
    if lay == 0:
        return ("w2", 0)
    b = (lay + 1) // 2
    if lay % 2 == 1:
        return ("w1", b)
    return ("w2", b)


def build_nc(reps=1, f32r=False):
    nc = bacc.Bacc("TRN2", target_bir_lowering=False, debug=False)

    xdec_d = nc.declare_dram_parameter("xdec", [32, T * BC], F32, isOutput=False)
    ylast_d = nc.declare_dram_parameter("ylast", [1, BC], F32, isOutput=False)
    win_d = nc.declare_dram_parameter("win", [CIN, 3 * F], F32, isOutput=False)
    wres_d = nc.declare_dram_parameter("wres", [CIN, F], F32, isOutput=False)
    w1_d = nc.declare_dram_parameter("w1l", [F, 7 * 3 * F], F32, isOutput=False)
    w2_d = nc.declare_dram_parameter("w2l", [F, 8 * 3 * F], F32, isOutput=False)
    wd0t_d = nc.declare_dram_parameter("wd0t", [F, F], F32, isOutput=False)
    wd0b_d = nc.declare_dram_parameter("wd0b", [F, F], F32, isOutput=False)
    wd1_d = nc.declare_dram_parameter("wd1", [F, 64], F32, isOutput=False)
    wd2_d = nc.declare_dram_parameter("wd2", [64, 1], F32, isOutput=False)
    encl_d = nc.declare_dram_parameter("encl", [F, BC], F32, isOutput=False)
    biasE_d = nc.declare_dram_parameter("biasE", [F, NCOL], F32, isOutput=False)
    biasT_d = nc.declare_dram_parameter("biasT", [F, NCOL], F32, isOutput=False)
    ident_d = nc.declare_dram_parameter("ident", [F, F], F32, isOutput=False)
    out_d = nc.declare_dram_parameter("out", [1, T * BC], F32, isOutput=True)

    with TileContext(nc) as tc:
        with (
            tc.tile_pool(name="consts", bufs=1) as cpool,
            tc.tile_pool(name="steps", bufs=3) as spool,
            tc.tile_pool(name="ps", bufs=1, space="PSUM") as ppool,
        ):
            # ---- const SBUF tiles + input DMAs ----
            win_sb = cpool.tile([CIN, 3 * F], F32, name="win_sb")
            wres_sb = cpool.tile([CIN, F], F32, name="wres_sb")
            w1_sb = cpool.tile([F, 7 * 3 * F], F32, name="w1_sb")
            w2_sb = cpool.tile([F, 8 * 3 * F], F32, name="w2_sb")
            wd0t_sb = cpool.tile([F, F], F32, name="wd0t_sb")
            wd0b_sb = cpool.tile([F, F], F32, name="wd0b_sb")
            wd1_sb = cpool.tile([F, 64], F32, name="wd1_sb")
            wd2_sb = cpool.tile([64, 1], F32, name="wd2_sb")
            encl_sb = cpool.tile([F, BC], F32, name="encl_sb")
            biasE_sb = cpool.tile([F, NCOL], F32, name="biasE_sb")
            biasT_sb = cpool.tile([F, NCOL], F32, name="biasT_sb")
            ident_sb = cpool.tile([F, F], F32, name="ident_sb")
            ones_sb = cpool.tile([F, 1], F32, name="ones_sb")
            csblk = cpool.tile([F, NCOL], F32, name="csblk")
            corrE = cpool.tile([F, NCOL], F32, name="corrE")
            corrT = cpool.tile([F, NCOL], F32, name="corrT")
            encc = cpool.tile([F, BC], F32, name="encc")
            dummy = cpool.tile([1, 1], F32, name="dummy")
            cimm = cpool.tile([F, 2], F32, name="cimm")

            # persistent histories
            x_hist = cpool.tile([CIN, (T + 1) * BC], F32, name="x_hist")
            ctils = [cpool.tile([F, T * BC], F32, name=f"ctil{b}") for b in range(NB)]
            htils = [cpool.tile([F, T * BC], F32, name=f"htil{b}") for b in range(NB)]

            # chunk big weight DMAs to 128-col pieces: keeps the per-reader
            # semaphore wait count below the hardware matmul wait-slot limit
            for sb, dr in [
                (biasE_sb, biasE_d), (biasT_sb, biasT_d),
                (win_sb, win_d), (wres_sb, wres_d),
                (w1_sb, w1_d), (w2_sb, w2_d),
                (wd0t_sb, wd0t_d), (wd0b_sb, wd0b_d),
                (wd1_sb, wd1_d), (wd2_sb, wd2_d),
                (encl_sb, encl_d), (ident_sb, ident_d),
            ]:
                ncols = sb.shape[-1]
                for c0 in range(0, ncols, F):
                    c1 = min(c0 + F, ncols)
                    nc.sync.dma_start(sb[:, c0:c1], dr[:, c0:c1])
            nc.sync.dma_start(x_hist[0:32, 0:T * BC], xdec_d[:, :])
            nc.sync.dma_start(x_hist[32:33, 0:BC], ylast_d[:, :])

            # warm the Exp table during DMA wait
            nc.scalar.activation(dummy[:, :], biasE_sb[0:1, 0:1], AF.Exp)
            nc.vector.memset(ones_sb[:, :], 1.0)
            nc.vector.memset(csblk[:, :], 0.0)
            nc.vector.memset(cimm[:, 0:1], -1.0)
            nc.vector.memset(cimm[:, 1:2], -2.0)

            # PE warm-up touches: one tiny matmul per DMA'd chunk / memset
            # tile, so each later matmul carries at most ONE sync wait
            # (the hardware LDWEIGHTS struct has very few wait slots).
            pswu = ppool.tile([1, 1], F32, name="pswu", tag="psy", bufs=1)
            touch_list = []
            for sb in (win_sb, wres_sb, w1_sb, w2_sb, wd0t_sb, wd0b_sb,
                       wd1_sb, wd2_sb, encl_sb, biasE_sb, biasT_sb,
                       ident_sb):
                p = sb.shape[0]
                for c0 in range(0, sb.shape[-1], F):
                    touch_list.append(sb[0:p, c0:c0 + 1])
            touch_list.append(x_hist[0:32, 0:1])
            touch_list.append(x_hist[0:32, 128:129])
            touch_list.append(x_hist[32:33, 0:1])
            touch_list.append(ones_sb[:, 0:1])
            wu_sb = cpool.tile([F, 1], F32, name="wu_sb")
            for t_ap in touch_list:
                nc.tensor.matmul(pswu[:, :], t_ap, t_ap, start=True, stop=True)
                nc.vector.tensor_copy(wu_sb[0:t_ap.shape[0], :], t_ap)

            # ---- prologue: colsum corrections ----
            psc = ppool.tile([F, 48], F32, name="psc", tag="psA", bufs=2)
            for lay in range(NLAY):
                wkind, b = _lay_w(lay)
                wt = w1_sb if wkind == "w1" else w2_sb
                base = ((b - 1) if wkind == "w1" else b) * 3
                for m in range(3):
                    ks = list(range(2 - m, 3))
                    col = lay * 3 + m
                    for j, k in enumerate(ks):
                        nc.tensor.matmul(
                            psc[:, col:col + 1],
                            wt[:, (base + k) * F:(base + k + 1) * F],
                            ones_sb[:, :],
                            start=(j == 0), stop=(j == len(ks) - 1),
                        )
            nc.tensor.matmul(psc[:, 45:46], wd0t_sb[:, :], ones_sb[:, :], start=True, stop=True)
            nc.tensor.matmul(psc[0:64, 46:47], wd1_sb[:, :], ones_sb[:, :], start=True, stop=True)
            nc.tensor.matmul(psc[0:1, 47:48], wd2_sb[:, :], ones_sb[0:64, :], start=True, stop=True)
            nc.vector.tensor_copy(csblk[:, 0:46], psc[:, 0:46])
            nc.vector.tensor_copy(csblk[0:64, 46:47], psc[0:64, 46:47])
            nc.vector.tensor_copy(csblk[0:1, 47:48], psc[0:1, 47:48])

            nc.vector.tensor_tensor(corrE[:, :], biasE_sb[:, :], csblk[:, :], op=OP.subtract)
            nc.vector.tensor_tensor(corrT[:, :], biasT_sb[:, :], csblk[:, :], op=OP.subtract)

            # enc_contrib = wd0b.T @ enc_last.T   [F, BC], constant over steps
            pse = ppool.tile([F, BC], F32, name="pse", tag="psA", bufs=2)
            nc.tensor.matmul(pse[:, :], wd0b_sb[:, :], encl_sb[:, :], start=True, stop=True)
            nc.vector.tensor_copy(encc[:, :], pse[:, :])

            # ---- helper: conv ELU site  out = min(exp(z), max(z+1,1)),
            # z = psum + corr[col]  (corr folds bias and colsum shifts) ----
            def elu_site(ps, col, out_ap, p=F):
                e_t = spool.tile([F, BC], F32, name="e_t", tag="e_t")
                t_t = spool.tile([F, BC], F32, name="t_t", tag="t_t")
                nc.scalar.activation(e_t[0:p, :], ps[0:p, :], AF.Exp,
                                     bias=corrE[0:p, col:col + 1])
                nc.vector.tensor_scalar(t_t[0:p, :], ps[0:p, :],
                                        corrT[0:p, col:col + 1], 1.0,
                                        op0=OP.add, op1=OP.max)
                nc.vector.tensor_tensor(out_ap, e_t[0:p, :], t_t[0:p, :], op=OP.min)

            # ---- helper: residual ELU site.  z = a + b + off (immediate);
            # out = elu(z)+1.  s = a+b on DVE; exp(s+off); max(s+off+1, 1). ----
            def elu_res_site(a_ap, b_ap, off, out_ap):
                s_t = spool.tile([F, BC], F32, name="s_t", tag="s_t")
                e_t = spool.tile([F, BC], F32, name="e_t", tag="e_t")
                t_t = spool.tile([F, BC], F32, name="t_t", tag="t_t")
                nc.vector.tensor_tensor(s_t[:, :], a_ap, b_ap, op=OP.add)
                bias_ap = cimm[:, 0:1] if off == -1.0 else cimm[:, 1:2]
                nc.scalar.activation(e_t[:, :], s_t[:, :], AF.Exp, bias=bias_ap)
                if off + 1.0 == 0.0:
                    nc.vector.tensor_scalar(t_t[:, :], s_t[:, :], 1.0, None,
                                            op0=OP.max)
                else:
                    nc.vector.tensor_scalar(t_t[:, :], s_t[:, :], off + 1.0, 1.0,
                                            op0=OP.add, op1=OP.max)
                nc.vector.tensor_tensor(out_ap, e_t[:, :], t_t[:, :], op=OP.min)

            # ---- main autoregressive loop (fully unrolled) ----
            # Conv weight lookup: (sbuf tile, column base) per (kind, block).
            def wsl(kind, b, k):
                if kind == "c1":
                    if b == 0:
                        return win_sb[:, k * F:(k + 1) * F]
                    return w1_sb[:, ((b - 1) * 3 + k) * F:((b - 1) * 3 + k + 1) * F]
                return w2_sb[:, (b * 3 + k) * F:(b * 3 + k + 1) * F]

            def hist(kind, b):
                if kind == "c1":
                    return x_hist if b == 0 else htils[b - 1]
                return ctils[b]

            F32R = mybir.dt.float32r

            def mmop(ps, lhsT, rhs, start, stop):
                if f32r:
                    lhsT = lhsT.bitcast(F32R)
                    rhs = rhs.bitcast(F32R)
                nc.tensor.matmul(ps[:, :], lhsT, rhs, start=start, stop=stop)

            # The final tap (k=2, current position) sits on the critical
            # path; taps k<2 read only prior-step history, so they are
            # pre-issued one block ahead to run during ELU waits.
            def old_taps(ps, kind, b, i):
                d = DILS[b]
                ntaps = 1 + (i >= d) + (i >= 2 * d)
                h = hist(kind, b)
                for j, k in enumerate(range(3 - ntaps, 2)):
                    p = i - (2 - k) * d
                    mmop(ps, wsl(kind, b, k), h[:, p * BC:(p + 1) * BC],
                         (j == 0), False)
                return ntaps

            def last_tap(ps, kind, b, i, ntaps, rhs):
                mmop(ps, wsl(kind, b, 2), rhs, (ntaps == 1), True)

            for i in list(range(T)) * reps:
                # -- pre-issue at step start: everything not needing y_hat(i-1)
                ps0 = ppool.tile([F, BC], F32, name="ps0", tag="ps0", bufs=1)
                nc.tensor.matmul(ps0[:, :], ident_sb[:, :], encc[:, :],
                                 start=True, stop=False)
                psB = ppool.tile([F, BC], F32, name="psB", tag="psB", bufs=2)
                ntB = old_taps(psB, "c2", 0, i)
                psA_next = ppool.tile([F, BC], F32, name="psA", tag="psA", bufs=2)
                ntA_next = old_taps(psA_next, "c1", 1, i)

                # -- block 0: conv_in + res (need y_hat from prev step) --
                d = DILS[0]
                ntaps = 1 + (i >= d) + (i >= 2 * d)
                ks = list(range(3 - ntaps, 3))
                psA = ppool.tile([F, BC], F32, name="psA", tag="psA", bufs=2)
                for j, k in enumerate(ks):
                    p = i - (2 - k) * d
                    nc.tensor.matmul(psA[:, :],
                                     win_sb[:, k * F:(k + 1) * F],
                                     x_hist[:, p * BC:(p + 1) * BC],
                                     start=(j == 0), stop=(j == len(ks) - 1))
                psR = ppool.tile([F, BC], F32, name="psR", tag="psR", bufs=1)
                nc.tensor.matmul(psR[:, :], wres_sb[:, :],
                                 x_hist[:, i * BC:(i + 1) * BC],
                                 start=True, stop=True)
                res_sb = spool.tile([F, BC], F32, name="res_sb", tag="res_sb")
                nc.vector.tensor_scalar(res_sb[:, :], psR[:, :],
                                        corrE[:, 48:49], None, op0=OP.add)
                elu_site(psA, 49, ctils[0][:, i * BC:(i + 1) * BC])

                for b in range(NB):
                    # psB for this block already holds its old taps; finish it.
                    last_tap(psB, "c2", b, i, ntB,
                             ctils[b][:, i * BC:(i + 1) * BC])
                    psB_cur, ntB_cur = psB, ntB
                    # pre-issue next block's old taps (fill ELU-B wait)
                    if b + 1 < NB:
                        psB = ppool.tile([F, BC], F32, name="psB", tag="psB", bufs=2)
                        ntB = old_taps(psB, "c2", b + 1, i)
                    lay = 0 if b == 0 else 2 * b
                    c2_t = spool.tile([F, BC], F32, name="c2_t", tag="c2_t")
                    elu_site(psB_cur, lay * 3 + (ntB_cur - 1), c2_t[:, :])
                    if b == 0:
                        elu_res_site(c2_t[:, :], res_sb[:, :], -1.0,
                                     htils[0][:, i * BC:(i + 1) * BC])
                    else:
                        elu_res_site(c2_t[:, :],
                                     htils[b - 1][:, i * BC:(i + 1) * BC], -2.0,
                                     htils[b][:, i * BC:(i + 1) * BC])
                    if b + 1 < NB:
                        # conv1 of block b+1: finish on-path tap
                        psA_cur, ntA_cur = psA_next, ntA_next
                        last_tap(psA_cur, "c1", b + 1, i, ntA_cur,
                                 htils[b][:, i * BC:(i + 1) * BC])
                        if b + 2 < NB:
                            psA_next = ppool.tile([F, BC], F32, name="psA",
                                                  tag="psA", bufs=2)
                            ntA_next = old_taps(psA_next, "c1", b + 2, i)
                        lay = 2 * (b + 1) - 1
                        elu_site(psA_cur, lay * 3 + (ntA_cur - 1),
                                 ctils[b + 1][:, i * BC:(i + 1) * BC])

                # --- head ---
                nc.tensor.matmul(ps0[:, :], wd0t_sb[:, :],
                                 htils[NB - 1][:, i * BC:(i + 1) * BC],
                                 start=False, stop=True)
                o0_t = spool.tile([F, BC], F32, name="o0_t", tag="o0_t")
                elu_site(ps0, 45, o0_t[:, :])

                ps1 = ppool.tile([64, BC], F32, name="ps1", tag="ps1", bufs=1)
                nc.tensor.matmul(ps1[:, :], wd1_sb[:, :], o0_t[:, :],
                                 start=True, stop=True)
                o1_t = spool.tile([64, BC], F32, name="o1_t", tag="o1_t")
                elu_site(ps1, 46, o1_t[:, :], p=64)

                psy = ppool.tile([1, BC], F32, name="psy", tag="psy", bufs=1)
                nc.tensor.matmul(psy[:, :], wd2_sb[:, :], o1_t[:, :],
                                 start=True, stop=True)
                nc.vector.tensor_scalar(
                    x_hist[32:33, (i + 1) * BC:(i + 2) * BC], psy[:, :],
                    corrE[0:1, 47:48], None, op0=OP.add)

            nc.sync.dma_start(out_d[:, :], x_hist[32:33, BC:(T + 1) * BC])

    nc.compile()
    return nc


def prepare_in_maps(inputs):
    f32 = np.float32

    def a(x):
        return np.ascontiguousarray(np.asarray(x, f32))

    dec = a(inputs["data_decoder"])          # [B,T,32]
    ly = a(inputs["last_y"])                 # [B]
    enc = a(inputs["data_encoder"])          # [B,128,128]
    w_in = a(inputs["w_in"])                 # [3,33,128]
    b_in = a(inputs["b_in"])
    w_res = a(inputs["w_res"])               # [1,33,128]
    b_res = a(inputs["b_res"])
    w1 = a(inputs["w1"])                     # [7,3,128,128]
    b1 = a(inputs["b1"])
    w2 = a(inputs["w2"])                     # [8,3,128,128]
    b2 = a(inputs["b2"])
    wd0 = a(inputs["wd0"])                   # [256,128]
    bd0 = a(inputs["bd0"])
    wd1 = a(inputs["wd1"])                   # [128,64]
    bd1 = a(inputs["bd1"])
    wd2 = a(inputs["wd2"])                   # [64,1]
    bd2 = a(inputs["bd2"])

    win_l = np.ascontiguousarray(w_in.transpose(1, 0, 2).reshape(CIN, 3 * F))
    w1_l = np.ascontiguousarray(w1.transpose(2, 0, 1, 3).reshape(F, 7 * 3 * F))
    w2_l = np.ascontiguousarray(w2.transpose(2, 0, 1, 3).reshape(F, 8 * 3 * F))

    biasE = np.zeros((F, NCOL), f32)
    biasT = np.zeros((F, NCOL), f32)
    for lay in range(NLAY):
        if lay % 2 == 1:
            bb = b1[(lay + 1) // 2 - 1]
        else:
            bb = b2[lay // 2]
        for m in range(3):
            biasE[:, lay * 3 + m] = bb
            biasT[:, lay * 3 + m] = bb + 1.0
    biasE[:, 45], biasT[:, 45] = bd0, bd0 + 1.0
    biasE[:64, 46], biasT[:64, 46] = bd1, bd1 + 1.0
    biasE[0, 47] = bd2[0]
    biasE[:, 48] = b_res
    biasE[:, 49], biasT[:, 49] = b_in, b_in + 1.0

    ident = np.eye(F, dtype=f32)

    in_maps = []
    for c in range(NCORES):
        sl = slice(c * BC, (c + 1) * BC)
        # xdec[ch, i*BC + j] = dec[batch j, step i, ch]
        xdec = np.ascontiguousarray(dec[sl].transpose(2, 1, 0).reshape(32, T * BC))
        ylast = np.ascontiguousarray(ly[sl].reshape(1, BC))
        encl = np.ascontiguousarray(enc[sl, -1, :].T)       # [128, BC]
        in_maps.append({
            "xdec": xdec, "ylast": ylast,
            "win": win_l, "wres": np.ascontiguousarray(w_res[0]),
            "w1l": w1_l, "w2l": w2_l,
            "wd0t": np.ascontiguousarray(wd0[:F]),
            "wd0b": np.ascontiguousarray(wd0[F:]),
            "wd1": wd1, "wd2": wd2,
            "encl": encl, "biasE": biasE, "biasT": biasT,
            "ident": ident,
        })
    return in_maps


@functools.lru_cache(maxsize=1)
def _built_nc():
    return build_nc()


def kernel(**inputs) -> np.ndarray:
    global LAST_EXEC_NS
    nc = _built_nc()
    in_maps = prepare_in_maps(inputs)
    trace = bool(os.environ.get("KERNEL_TRACE"))
    try:
        r = run_bass_kernel_spmd(nc, in_maps, list(range(NCORES)), trace=trace)
    except ModuleNotFoundError:
        r = run_bass_kernel_spmd(nc, in_maps, list(range(NCORES)), trace=False)
    LAST_EXEC_NS = r.exec_time_ns if r.exec_time_ns else r.mean_exec_time_ns
    outs = []
    for c in range(NCORES):
        o = np.asarray(r.results[c]["out"]).reshape(T, BC).T   # [BC, T]
        outs.append(o)
    return np.ascontiguousarray(np.concatenate(outs, axis=0).astype(np.float32))
